# revision 1
# baseline (speedup 1.0000x reference)
"""Trainium2 Bass kernel for nn_BehlerG2 (Behler-style angular symmetry functions).

Strategy:
- 8 cores; core c handles batch b = c // 2, atom half h = c % 2 (128 atoms/core,
  one atom per SBUF partition).
- Host compacts each atom's triple list by mask (mask==0 triples contribute
  exactly 0), pads to a uniform length Tp with a dummy table entry (index 256,
  all-zero fields incl. atomic number -> exact 0 contribution).
- Device gathers per-triple neighbor fields (x, y, z, |p|^2, z_num for j and k)
  with a GPSIMD ap_gather from a per-partition field table, relayouts to
  atom-major tiles via SBUF->SBUF DMA, then computes distances / cutoffs /
  angular terms on DVE+ACT and reduces with fused tensor_tensor_reduce.
- Everything is fp32; formulas mirror the reference (incl. the +1e-12 epsilon).
"""

import sys

if "/opt/trn_rl_repo" not in sys.path:
    sys.path.insert(0, "/opt/trn_rl_repo")

import numpy as np

import concourse.bacc as bacc
import concourse.mybir as mybir
import concourse.tile as tile
from concourse.alu_op_type import AluOpType as alu
from concourse.bass_utils import run_bass_kernel_spmd

f32 = mybir.dt.float32
i16 = mybir.dt.int16

B, A, T = 4, 256, 512
NCORES = 8
P = 128          # atoms per core == partitions
NE = 257         # table entries (256 atoms + 1 dummy)
ZETAS = np.array([1.0, 2.0, 4.0, 8.0], dtype=np.float64)
CUTOFF = 6.0
PI = float(np.pi)

AF = mybir.ActivationFunctionType


def _build_program(Tp: int, etas: np.ndarray):
    """Build the SPMD Bass program for per-core tiles of [128 atoms, Tp triples]."""
    nc = bacc.Bacc("TRN2", target_bir_lowering=False, debug=False, num_devices=NCORES)

    NI = 16 * 2 * Tp  # ap_gather num_idxs per 16-partition group

    tab_d = nc.dram_tensor("tab", [P, NE], f32, kind="ExternalInput")
    idx_d = nc.dram_tensor("idx", [P, NI // 16], i16, kind="ExternalInput")
    scal_d = nc.dram_tensor("scal", [P, 4], f32, kind="ExternalInput")
    clo_d = nc.dram_tensor("clo", [P, 32], f32, kind="ExternalInput")
    chi_d = nc.dram_tensor("chi", [P, 32], f32, kind="ExternalInput")
    out_d = nc.dram_tensor("out", [P, 64], f32, kind="ExternalOutput")

    with tile.TileContext(nc) as tc:
        with tc.tile_pool(name="main", bufs=1) as pool:
            TAB = pool.tile([P, NE], f32)
            nc.sync.dma_start(TAB, tab_d.ap())
            IDX = pool.tile([P, NI // 16], i16)
            nc.sync.dma_start(IDX, idx_d.ap())
            SCAL = pool.tile([P, 4], f32)
            nc.sync.dma_start(SCAL, scal_d.ap())
            CLO = pool.tile([P, 32], f32)
            nc.sync.dma_start(CLO, clo_d.ap())
            CHI = pool.tile([P, 32], f32)
            nc.sync.dma_start(CHI, chi_d.ap())

            # --- gather ---
            G = pool.tile([P, NI], f32)
            nc.gpsimd.ap_gather(G, TAB, IDX, channels=P, num_elems=NE, d=1, num_idxs=NI)

            # --- relayout to atom-major field tiles via DRAM round-trip ---
            # (direct SBUF->SBUF repartition DMAs fault the exec unit)
            with tc.tile_pool(name="dram", bufs=1, space="DRAM") as dpool:
                GD = dpool.tile([P, NI], f32)
                nc.sync.dma_start(GD, G)
                FLD = pool.tile([P, 5, 2, Tp], f32)
                FLDv = FLD.rearrange("p f jk s -> p f (jk s)")
                GDr = GD.rearrange("(g q) (a e) -> g q a e", q=16, a=16)
                for f in range(5):
                    nc.sync.dma_start(FLDv[:, f], GDr[:, f])

            xj, xk = FLD[:, 0, 0], FLD[:, 0, 1]
            yj, yk = FLD[:, 1, 0], FLD[:, 1, 1]
            zj, zk = FLD[:, 2, 0], FLD[:, 2, 1]
            n2j, n2k = FLD[:, 3, 0], FLD[:, 3, 1]
            znj, znk = FLD[:, 4, 0], FLD[:, 4, 1]
            xi, yi, zi, n2i = (SCAL[:, q : q + 1] for q in range(4))

            # --- squared distances from true coordinate differences ---
            R2 = pool.tile([P, 3, Tp], f32)  # [rij2, rik2, rjk2]
            t1 = pool.tile([P, Tp], f32)
            t2 = pool.tile([P, Tp], f32)
            t3 = pool.tile([P, Tp], f32)
            DJ = pool.tile([P, 3, Tp], f32)  # pos_j - pos_i
            DK = pool.tile([P, 3, Tp], f32)  # pos_k - pos_i

            for side, (D, (xs, ys, zs)) in enumerate(((DJ, (xj, yj, zj)), (DK, (xk, yk, zk)))):
                for q, (vs, vi) in enumerate(((xs, xi), (ys, yi), (zs, zi))):
                    nc.vector.tensor_scalar(out=D[:, q], in0=vs, scalar1=vi, scalar2=None, op0=alu.subtract)
                nc.vector.tensor_tensor(out=t1, in0=D[:, 0], in1=D[:, 0], op=alu.mult)
                nc.vector.tensor_tensor(out=t2, in0=D[:, 1], in1=D[:, 1], op=alu.mult)
                nc.vector.tensor_tensor(out=t1, in0=t1, in1=t2, op=alu.add)
                nc.vector.tensor_tensor(out=t2, in0=D[:, 2], in1=D[:, 2], op=alu.mult)
                nc.vector.tensor_tensor(out=R2[:, side], in0=t1, in1=t2, op=alu.add)

            # r_jk^2 from (pos_j - pos_i) - (pos_k - pos_i) = pos_j - pos_k
            nc.vector.tensor_tensor(out=t1, in0=xj, in1=xk, op=alu.subtract)
            nc.vector.tensor_tensor(out=t1, in0=t1, in1=t1, op=alu.mult)
            nc.vector.tensor_tensor(out=t2, in0=yj, in1=yk, op=alu.subtract)
            nc.vector.tensor_tensor(out=t2, in0=t2, in1=t2, op=alu.mult)
            nc.vector.tensor_tensor(out=t1, in0=t1, in1=t2, op=alu.add)
            nc.vector.tensor_tensor(out=t2, in0=zj, in1=zk, op=alu.subtract)
            nc.vector.tensor_tensor(out=t2, in0=t2, in1=t2, op=alu.mult)
            nc.vector.tensor_tensor(out=R2[:, 2], in0=t1, in1=t2, op=alu.add)

            # --- bias constants for ACT (must be APs) ---
            EPS = pool.tile([P, 1], f32)
            nc.vector.memset(EPS, 1e-12)
            HPI = pool.tile([P, 1], f32)
            nc.vector.memset(HPI, PI / 2.0)

            # --- r = sqrt(r2 + 1e-12) ---
            R = pool.tile([P, 3, Tp], f32)
            R2flat = R2.rearrange("p a s -> p (a s)")
            nc.scalar.activation(R.rearrange("p a s -> p (a s)"), R2flat, AF.Sqrt, bias=EPS)

            # reference uses r**2 (the squared epsilon-ed sqrt) everywhere downstream;
            # matching that exactly matters for degenerate (self-neighbor) triples
            SQ2 = pool.tile([P, 3, Tp], f32)
            nc.vector.tensor_tensor(
                out=SQ2.rearrange("p a s -> p (a s)"),
                in0=R.rearrange("p a s -> p (a s)"),
                in1=R.rearrange("p a s -> p (a s)"),
                op=alu.mult,
            )
            rij2, rik2, rjk2 = SQ2[:, 0], SQ2[:, 1], SQ2[:, 2]

            # --- S3 = rij2+rik2+rjk2, num = rij2+rik2-rjk2 ---
            S = pool.tile([P, Tp], f32)
            S3 = pool.tile([P, Tp], f32)
            NUM = pool.tile([P, Tp], f32)
            nc.vector.tensor_tensor(out=S, in0=rij2, in1=rik2, op=alu.add)
            nc.vector.tensor_tensor(out=S3, in0=S, in1=rjk2, op=alu.add)
            nc.vector.tensor_tensor(out=NUM, in0=S, in1=rjk2, op=alu.subtract)

            # --- cutoff: fc(r) = cos(pi r / 12)^2 masked by (r2 < 36) ---
            # cos(x) = sin(x + pi/2); product of the three cosines, then square.
            C3 = pool.tile([P, 3, Tp], f32)
            RCLAMP = pool.tile([P, 3, Tp], f32)
            nc.vector.tensor_scalar(
                out=RCLAMP.rearrange("p a s -> p (a s)"),
                in0=R.rearrange("p a s -> p (a s)"),
                scalar1=CUTOFF,
                scalar2=None,
                op0=alu.min,
            )
            nc.scalar.activation(
                C3.rearrange("p a s -> p (a s)"),
                RCLAMP.rearrange("p a s -> p (a s)"),
                AF.Sin,
                scale=PI / 12.0,
                bias=HPI,
            )
            nc.vector.tensor_tensor(out=t1, in0=C3[:, 0], in1=C3[:, 1], op=alu.mult)
            nc.vector.tensor_tensor(out=t1, in0=t1, in1=C3[:, 2], op=alu.mult)
            CSQ = pool.tile([P, Tp], f32)
            nc.scalar.activation(CSQ, t1, AF.Square)

            nc.vector.tensor_tensor(out=t2, in0=rij2, in1=rik2, op=alu.max)
            nc.vector.tensor_tensor(out=t2, in0=t2, in1=rjk2, op=alu.max)
            nc.vector.tensor_scalar(out=t2, in0=t2, scalar1=CUTOFF * CUTOFF, scalar2=None, op0=alu.is_lt)

            # base = csq * mask * (znj * znk)
            W = pool.tile([P, Tp], f32)
            nc.vector.tensor_tensor(out=W, in0=znj, in1=znk, op=alu.mult)
            BASE = pool.tile([P, Tp], f32)
            nc.vector.tensor_tensor(out=t3, in0=CSQ, in1=t2, op=alu.mult)
            nc.vector.tensor_tensor(out=BASE, in0=t3, in1=W, op=alu.mult)

            # --- cos(theta) = 0.5 * num / (rij * rik) ---
            COS = pool.tile([P, Tp], f32)
            RR = pool.tile([P, Tp], f32)
            nc.vector.tensor_tensor(out=RR, in0=R[:, 0], in1=R[:, 1], op=alu.mult)
            nc.vector.reciprocal(out=RR, in_=RR)
            nc.vector.scalar_tensor_tensor(out=COS, in0=NUM, scalar=0.5, in1=RR, op0=alu.mult, op1=alu.mult)
            U = pool.tile([P, 4, Tp], f32)  # u, u2, u4, u8 (pre-multiplied by BASE later)
            UPOW = pool.tile([P, 4, Tp], f32)
            nc.scalar.activation(UPOW[:, 0], COS, AF.Copy, scale=-1.0, bias=1.0)
            nc.scalar.activation(UPOW[:, 1], UPOW[:, 0], AF.Square)
            nc.scalar.activation(UPOW[:, 2], UPOW[:, 1], AF.Square)
            nc.scalar.activation(UPOW[:, 3], UPOW[:, 2], AF.Square)
            for z in range(4):
                nc.vector.tensor_tensor(out=U[:, z], in0=UPOW[:, z], in1=BASE, op=alu.mult)

            # --- radial exponentials ---
            ET = pool.tile([P, 8, Tp], f32)
            for e in range(8):
                nc.scalar.activation(ET[:, e], S3, AF.Exp, scale=float(-etas[e]))

            # --- 32 multiply + reduce pairs ---
            # (accum_out-style fused reduce instructions fault the exec unit
            #  in this environment; plain tensor_reduce works)
            PART = pool.tile([P, 32], f32)
            scratch = pool.tile([P, Tp], f32)
            for e in range(8):
                for z in range(4):
                    nc.vector.tensor_tensor(out=scratch, in0=ET[:, e], in1=U[:, z], op=alu.mult)
                    nc.vector.tensor_reduce(
                        out=PART[:, e * 4 + z : e * 4 + z + 1],
                        in_=scratch,
                        axis=mybir.AxisListType.X,
                        op=alu.add,
                    )

            # --- final scaling into [128, 64] ---
            OUT = pool.tile([P, 64], f32)
            Ov = OUT.rearrange("p (e g z) -> p e g z", e=8, g=2, z=4)
            Pv = PART.rearrange("p (e z) -> p e z", e=8, z=4)
            Lv = CLO.rearrange("p (e z) -> p e z", e=8, z=4)
            Hv = CHI.rearrange("p (e z) -> p e z", e=8, z=4)
            nc.vector.tensor_tensor(out=Ov[:, :, 0], in0=Pv, in1=Lv, op=alu.mult)
            nc.vector.tensor_tensor(out=Ov[:, :, 1], in0=Pv, in1=Hv, op=alu.mult)
            nc.sync.dma_start(out_d.ap(), OUT)

    nc.compile()
    return nc


def _prepare_host(inputs):
    positions = np.asarray(inputs["positions"], dtype=np.float32)
    nj = np.asarray(inputs["neighbors_j"])
    nk = np.asarray(inputs["neighbors_k"])
    mask = np.asarray(inputs["mask_triples"]) != 0
    atomic = np.asarray(inputs["atomic_numbers"]).astype(np.float32)
    etas = np.asarray(inputs["etas"], dtype=np.float32)

    counts = mask.sum(axis=2)  # [B, A]
    Tp = int(counts.max())
    Tp = max(8, ((Tp + 7) // 8) * 8)

    # per-atom padded (j, k) index lists
    jpad = np.full((B, A, Tp), NE - 1, dtype=np.int16)
    kpad = np.full((B, A, Tp), NE - 1, dtype=np.int16)
    for b in range(B):
        for a in range(A):
            m = mask[b, a]
            c = int(counts[b, a])
            jpad[b, a, :c] = nj[b, a][m]
            kpad[b, a, :c] = nk[b, a][m]

    in_maps = []
    zeta = ZETAS
    clo_row = np.array([2.0 ** (1.0 - zeta[z]) for _ in range(8) for z in range(4)], dtype=np.float32)
    chi_row = np.array([2.0 ** (1.0 + zeta[z]) for _ in range(8) for z in range(4)], dtype=np.float32)
    clo = np.broadcast_to(clo_row, (P, 32)).copy()
    chi = np.broadcast_to(chi_row, (P, 32)).copy()

    for c in range(NCORES):
        b, h = divmod(c, 2)
        asl = slice(h * P, (h + 1) * P)
        pos_b = positions[b]  # [256, 3]
        n2_b = (pos_b * pos_b).sum(axis=1)
        # field table: partition 16g+f holds field f; entry 256 is all-zero dummy
        tab = np.zeros((P, NE), np.float32)
        fields = np.zeros((5, NE), np.float32)
        fields[0, :A] = pos_b[:, 0]
        fields[1, :A] = pos_b[:, 1]
        fields[2, :A] = pos_b[:, 2]
        fields[3, :A] = n2_b
        fields[4, :A] = atomic[b]
        for g in range(8):
            tab[16 * g : 16 * g + 5] = fields
        # gather index lists, wrapped per 16-partition group
        idx = np.zeros((P, 2 * Tp), np.int16)
        for g in range(8):
            atoms = h * P + 16 * g + np.arange(16)
            L = np.concatenate(
                [np.stack([jpad[b, a], kpad[b, a]]).reshape(-1) for a in atoms]
            )  # [16 * 2 * Tp]
            idx[16 * g : 16 * g + 16] = L.reshape(-1, 16).T
        # per-atom scalars
        scal = np.zeros((P, 4), np.float32)
        scal[:, 0:3] = pos_b[asl]
        scal[:, 3] = n2_b[asl]
        in_maps.append({"tab": tab, "idx": idx, "scal": scal, "clo": clo, "chi": chi})

    return Tp, etas, in_maps


def kernel(**inputs) -> np.ndarray:
    Tp, etas, in_maps = _prepare_host(inputs)
    nc = _build_program(Tp, etas)
    res = run_bass_kernel_spmd(nc, in_maps, core_ids=list(range(NCORES)))
    out = np.zeros((B, A, 64), np.float32)
    for c in range(NCORES):
        b, h = divmod(c, 2)
        out[b, h * P : (h + 1) * P] = res.results[c]["out"]
    return out



# revision 4
# speedup vs baseline: 6.6546x; 6.6546x over previous
"""Trainium2 Bass kernel for nn_BehlerG2 (Behler-style angular symmetry functions).

Strategy:
- 8 cores; core c handles batch b = c // 2, atom half h = c % 2 (128 atoms/core,
  one atom per SBUF partition, Tp compacted triples along the free axis).
- Host compacts each atom's triple list by mask (mask==0 triples contribute
  exactly 0) and gathers the neighbor fields (pure data movement: coords of
  j/k and the two atomic numbers) into a dense [128, 8, Tp] tile per core.
- Device does all arithmetic: distances, cosine cutoffs, and the 8x4
  (eta x zeta) radial/angular cross products.  The angular power and the
  cutoff/weight product are evaluated in log space,
      u^zeta * B = exp(zeta*(ln V - ln RR2) + 2*ln CP + ln W),
  which avoids the slow DVE reciprocal and the pow chain.  The 32 (e,z)
  multiply+reduce pairs run as fused bf16 tensor_tensor_reduce instructions
  (bf16 inputs -> 2x DVE rate; fp32 accumulator).
- ACT ops are grouped by activation-table set (Square/Sqrt -> Sin -> Ln/Exp)
  so only 3 table loads are required.
"""

import sys

if "/opt/trn_rl_repo" not in sys.path:
    sys.path.insert(0, "/opt/trn_rl_repo")

import numpy as np

import concourse.bacc as bacc
import concourse.mybir as mybir
import concourse.tile as tile
from concourse.alu_op_type import AluOpType as alu
from concourse.bass_utils import run_bass_kernel_spmd

f32 = mybir.dt.float32
bf16 = mybir.dt.bfloat16

B, A, T = 4, 256, 512
NCORES = 8
P = 128          # atoms per core == partitions
ZETAS = np.array([1.0, 2.0, 4.0, 8.0], dtype=np.float64)
CUTOFF = 6.0
PI = float(np.pi)
LNFLOOR = 1e-30  # clamp floor before Ln so padding/degenerate triples hit -69, not NaN

AF = mybir.ActivationFunctionType


def _build_program(Tp: int, etas: np.ndarray):
    """Build the SPMD Bass program for per-core tiles of [128 atoms, Tp triples]."""
    nc = bacc.Bacc("TRN2", target_bir_lowering=False, debug=False, num_devices=NCORES)

    f_d = nc.dram_tensor("f", [P, 8 * Tp], f32, kind="ExternalInput")
    scal_d = nc.dram_tensor("scal", [P, 4], f32, kind="ExternalInput")
    clo_d = nc.dram_tensor("clo", [P, 32], f32, kind="ExternalInput")
    chi_d = nc.dram_tensor("chi", [P, 32], f32, kind="ExternalInput")
    out_d = nc.dram_tensor("out", [P, 64], f32, kind="ExternalOutput")

    with tile.TileContext(nc) as tc:
        with tc.tile_pool(name="main", bufs=1) as pool:
            F = pool.tile([P, 8, Tp], f32)
            nc.sync.dma_start(F.rearrange("p f t -> p (f t)"), f_d.ap())
            SCAL = pool.tile([P, 4], f32)
            nc.sync.dma_start(SCAL, scal_d.ap())
            CLO = pool.tile([P, 32], f32)
            nc.sync.dma_start(CLO, clo_d.ap())
            CHI = pool.tile([P, 32], f32)
            nc.sync.dma_start(CHI, chi_d.ap())

            # --- constants (ACT bias operands must be APs) ---
            EPS = pool.tile([P, 1], f32)
            nc.vector.memset(EPS, 1e-12)
            HPI = pool.tile([P, 1], f32)
            nc.vector.memset(HPI, PI / 2.0)

            # --- W = znj * znk on Pool (only needs F) ---
            W = pool.tile([P, Tp], f32)
            nc.gpsimd.tensor_tensor(out=W, in0=F[:, 6], in1=F[:, 7], op=alu.mult)

            # --- coordinate differences: D9 = [dj(x,y,z), dk(x,y,z), djk(x,y,z)] ---
            D9 = pool.tile([P, 9, Tp], f32)
            for c in range(6):
                nc.vector.tensor_scalar(
                    out=D9[:, c], in0=F[:, c],
                    scalar1=SCAL[:, c % 3 : c % 3 + 1], scalar2=None,
                    op0=alu.subtract,
                )
            nc.vector.tensor_tensor(out=D9[:, 6:9], in0=D9[:, 0:3], in1=D9[:, 3:6], op=alu.subtract)

            # --- squared distances ---
            SQ9 = pool.tile([P, 9, Tp], f32)
            nc.scalar.activation(
                SQ9.rearrange("p f t -> p (f t)"),
                D9.rearrange("p f t -> p (f t)"),
                AF.Square,
            )
            SQv = SQ9.rearrange("p (d c) t -> p d c t", d=3)
            R2 = pool.tile([P, 3, Tp], f32)  # [rij2, rik2, rjk2]
            nc.vector.tensor_tensor(out=R2, in0=SQv[:, :, 0], in1=SQv[:, :, 1], op=alu.add)
            nc.vector.tensor_tensor(out=R2, in0=R2, in1=SQv[:, :, 2], op=alu.add)

            # --- r = sqrt(r2 + 1e-12) ---
            R = pool.tile([P, 3, Tp], f32)
            nc.scalar.activation(
                R.rearrange("p a t -> p (a t)"),
                R2.rearrange("p a t -> p (a t)"),
                AF.Sqrt,
                bias=EPS,
            )

            # --- cutoff cosines: c = cos(pi*min(r,6)/12) = sin(pi/12*rc + pi/2) ---
            RC = pool.tile([P, 3, Tp], f32)
            nc.vector.tensor_scalar(
                out=RC.rearrange("p a t -> p (a t)"),
                in0=R.rearrange("p a t -> p (a t)"),
                scalar1=CUTOFF, scalar2=None, op0=alu.min,
            )
            C3 = pool.tile([P, 3, Tp], f32)
            nc.scalar.activation(
                C3.rearrange("p a t -> p (a t)"),
                RC.rearrange("p a t -> p (a t)"),
                AF.Sin,
                scale=PI / 12.0,
                bias=HPI,
            )
            # CP = c_ij * c_ik * c_jk on Pool
            CP = pool.tile([P, Tp], f32)
            nc.gpsimd.tensor_tensor(out=CP, in0=C3[:, 0], in1=C3[:, 1], op=alu.mult)
            nc.gpsimd.tensor_tensor(out=CP, in0=CP, in1=C3[:, 2], op=alu.mult)

            # --- scalar combinations of squared distances ---
            S = pool.tile([P, Tp], f32)
            S3 = pool.tile([P, Tp], f32)
            NUM = pool.tile([P, Tp], f32)
            nc.vector.tensor_tensor(out=S, in0=R2[:, 0], in1=R2[:, 1], op=alu.add)
            nc.vector.tensor_tensor(out=S3, in0=S, in1=R2[:, 2], op=alu.add)
            nc.vector.tensor_tensor(out=NUM, in0=S, in1=R2[:, 2], op=alu.subtract)

            # --- V = 2*rij*rik - NUM  (so 1 - cos_theta = V / (2*rij*rik)) ---
            RR2 = pool.tile([P, Tp], f32)
            nc.vector.scalar_tensor_tensor(
                out=RR2, in0=R[:, 0], scalar=2.0, in1=R[:, 1], op0=alu.mult, op1=alu.mult
            )
            V = pool.tile([P, Tp], f32)
            nc.vector.tensor_tensor(out=V, in0=RR2, in1=NUM, op=alu.subtract)
            # clamps before Ln (padding triples / degenerate angles)
            nc.vector.tensor_scalar(out=V, in0=V, scalar1=LNFLOOR, scalar2=None, op0=alu.max)
            nc.vector.tensor_scalar(out=CP, in0=CP, scalar1=LNFLOOR, scalar2=None, op0=alu.max)
            nc.vector.tensor_scalar(out=W, in0=W, scalar1=LNFLOOR, scalar2=None, op0=alu.max)

            # --- logs ---
            LV = pool.tile([P, Tp], f32)
            LR = pool.tile([P, Tp], f32)
            LC = pool.tile([P, Tp], f32)
            LW = pool.tile([P, Tp], f32)
            nc.scalar.activation(LV, V, AF.Ln)
            nc.scalar.activation(LR, RR2, AF.Ln)
            nc.scalar.activation(LC, CP, AF.Ln)
            nc.scalar.activation(LW, W, AF.Ln)

            # g = ln(1-cos_theta) = LV - LR ; h = ln(CP^2 * W) = 2*LC + LW
            G0 = pool.tile([P, Tp], f32)
            H = pool.tile([P, Tp], f32)
            nc.vector.tensor_tensor(out=G0, in0=LV, in1=LR, op=alu.subtract)
            nc.vector.scalar_tensor_tensor(
                out=H, in0=LC, scalar=2.0, in1=LW, op0=alu.mult, op1=alu.add
            )
            # G_z = zeta_z * g + h
            G = pool.tile([P, 4, Tp], f32)
            for z in range(4):
                nc.vector.scalar_tensor_tensor(
                    out=G[:, z], in0=G0, scalar=float(ZETAS[z]), in1=H,
                    op0=alu.mult, op1=alu.add,
                )

            # --- exponentials (bf16 outputs feed the product stage at 2x) ---
            ETb = pool.tile([P, 8, Tp], bf16)
            for e in range(8):
                nc.scalar.activation(ETb[:, e], S3, AF.Exp, scale=float(-etas[e]))
            UBb = pool.tile([P, 4, Tp], bf16)
            for z in range(4):
                nc.scalar.activation(UBb[:, z], G[:, z], AF.Exp)

            # --- 32 fused multiply+reduce pairs ---
            # (InstTensorTensorReduce faults the exec unit on this HW; the
            #  scalar_tensor_tensor accum_out path does the same fusion and works)
            PART = pool.tile([P, 32], f32)
            PS = [pool.tile([P, Tp], bf16, name=f"ps{i}") for i in range(4)]
            for e in range(8):
                for z in range(4):
                    j = e * 4 + z
                    nc.vector.scalar_tensor_tensor(
                        out=PS[j % 4],
                        in0=ETb[:, e],
                        scalar=1.0,
                        in1=UBb[:, z],
                        op0=alu.mult,
                        op1=alu.mult,
                        accum_out=PART[:, j : j + 1],
                    )

            # --- final scaling into [128, 64] ---
            OUT = pool.tile([P, 64], f32)
            Ov = OUT.rearrange("p (e g z) -> p e g z", e=8, g=2, z=4)
            Pv = PART.rearrange("p (e z) -> p e z", e=8, z=4)
            Lv = CLO.rearrange("p (e z) -> p e z", e=8, z=4)
            Hv = CHI.rearrange("p (e z) -> p e z", e=8, z=4)
            nc.vector.tensor_tensor(out=Ov[:, :, 0], in0=Pv, in1=Lv, op=alu.mult)
            nc.vector.tensor_tensor(out=Ov[:, :, 1], in0=Pv, in1=Hv, op=alu.mult)
            nc.sync.dma_start(out_d.ap(), OUT)

    nc.compile()
    return nc


def _prepare_host(inputs):
    positions = np.asarray(inputs["positions"], dtype=np.float32)
    nj = np.asarray(inputs["neighbors_j"])
    nk = np.asarray(inputs["neighbors_k"])
    mask = np.asarray(inputs["mask_triples"]) != 0
    atomic = np.asarray(inputs["atomic_numbers"]).astype(np.float32)
    etas = np.asarray(inputs["etas"], dtype=np.float32)

    counts = mask.sum(axis=2)  # [B, A]
    Tp = int(counts.max())
    Tp = max(16, ((Tp + 15) // 16) * 16)

    # stable-sort valid triples to the front, take the first Tp slots
    order = np.argsort(~mask, axis=2, kind="stable")[:, :, :Tp]
    jc = np.take_along_axis(nj, order, axis=2)  # [B, A, Tp]
    kc = np.take_along_axis(nk, order, axis=2)
    valid = np.take_along_axis(mask, order, axis=2)

    bidx = np.arange(B)[:, None, None]
    pj = positions[bidx, jc]  # [B, A, Tp, 3]
    pk = positions[bidx, kc]
    znj = atomic[bidx, jc] * valid  # zero -> padding contributes exactly 0
    znk = atomic[bidx, kc]

    # field tile [B, A, 8, Tp]: xj yj zj xk yk zk znj znk
    F = np.empty((B, A, 8, Tp), np.float32)
    F[:, :, 0:3] = np.moveaxis(pj, 3, 2)
    F[:, :, 3:6] = np.moveaxis(pk, 3, 2)
    F[:, :, 6] = znj
    F[:, :, 7] = znk

    zeta = ZETAS
    clo_row = np.array([2.0 ** (1.0 - zeta[z]) for _ in range(8) for z in range(4)], dtype=np.float32)
    chi_row = np.array([2.0 ** (1.0 + zeta[z]) for _ in range(8) for z in range(4)], dtype=np.float32)
    clo = np.broadcast_to(clo_row, (P, 32)).copy()
    chi = np.broadcast_to(chi_row, (P, 32)).copy()

    in_maps = []
    for c in range(NCORES):
        b, h = divmod(c, 2)
        asl = slice(h * P, (h + 1) * P)
        scal = np.zeros((P, 4), np.float32)
        scal[:, 0:3] = positions[b, asl]
        in_maps.append({
            "f": np.ascontiguousarray(F[b, asl].reshape(P, 8 * Tp)),
            "scal": scal,
            "clo": clo,
            "chi": chi,
        })

    return Tp, etas, in_maps


def kernel(**inputs) -> np.ndarray:
    Tp, etas, in_maps = _prepare_host(inputs)
    nc = _build_program(Tp, etas)
    res = run_bass_kernel_spmd(nc, in_maps, core_ids=list(range(NCORES)))
    out = np.zeros((B, A, 64), np.float32)
    for c in range(NCORES):
        b, h = divmod(c, 2)
        out[b, h * P : (h + 1) * P] = res.results[c]["out"]
    return out
